# revision 15
# baseline (speedup 1.0000x reference)
"""Trainium2 Bass kernel for nn_Decoder (copy-mechanism decoder step), 8-core SPMD.

Strategy (see DESIGN.md):
- Vocab-shard Wg (host-pretransposed, bias folded as extra matmul row) across 8 cores;
  batch-shard encoder_outputs (host-pretransposed to [2H, 16*T] bf16).
- GRU replicated on every core (tiny).
- Two NEFF executions: NEFF-1 computes exp(score_g) slab + exp(score_c) + partial sums;
  host concatenates the tiny cross-core pieces (pure data movement); NEFF-2 computes the
  softmax denominator, normalizes the slab, applies the copy-distribution scatter via
  gpsimd local_scatter (per-partition = per-batch-row), and the selective-read output.
- Host only does integer index bookkeeping + layout transforms; all float math that
  the reference performs on tensors happens on device.
"""

import numpy as np
import ml_dtypes

import concourse.bass as bass
import concourse.mybir as mybir
import concourse.tile as tile
from concourse import bacc
from concourse.bass_utils import run_bass_kernel_spmd

B, T, V, H, E, OOV = 128, 200, 50257, 256, 128, 50
NCORE = 8
BS = B // NCORE          # 16 rows per core
VS = 6292                # per-core padded vocab shard; 8*6292 = 50336 >= V+OOV
VPAD = NCORE * VS
BT = BS * T              # 3200
# score_g chunks (free-dim) for matmul/exp pipeline
SG_CHUNKS = [512] * 12 + [148]
assert sum(SG_CHUNKS) == VS
# local_scatter chunks (num_elems must be even, < 2048)
LS_CHUNKS = [1574, 1574, 1574, 1570]
assert sum(LS_CHUNKS) == VS and all(c % 2 == 0 for c in LS_CHUNKS)
# enc-proj bt chunks
EPT_CHUNKS = [512] * 6 + [128]
assert sum(EPT_CHUNKS) == BT
MPAD = 8                 # max matched (b,t) pairs per core for selective read
OOV_SL = (V - 7 * VS, V - 7 * VS + 64)  # slice of core-7's slab containing OOV cols

F32, F32R, BF16, I16, I32 = (
    mybir.dt.float32, mybir.dt.float32r, mybir.dt.bfloat16, mybir.dt.int16, mybir.dt.int32,
)

_CACHE = {}


def _build_neff1():
    nc = bacc.Bacc("TRN2", target_bir_lowering=False, debug=False, num_devices=NCORE)
    wgt = nc.dram_tensor("wgt", [257, VS], F32R, kind="ExternalInput")
    w_ihT = nc.dram_tensor("w_ihT", [640, 768], F32R, kind="ExternalInput")
    w_hhT = nc.dram_tensor("w_hhT", [256, 768], F32R, kind="ExternalInput")
    b_ih = nc.dram_tensor("b_ih", [1, 768], F32, kind="ExternalInput")
    b_hh = nc.dram_tensor("b_hh", [1, 768], F32, kind="ExternalInput")
    selT = nc.dram_tensor("selT", [512, 128], F32R, kind="ExternalInput")
    prevT = nc.dram_tensor("prevT", [256, 128], F32R, kind="ExternalInput")
    prev_n = nc.dram_tensor("prev_n", [128, 256], F32, kind="ExternalInput")
    emb = nc.dram_tensor("emb", [V, E], F32, kind="ExternalInput")
    eoff = nc.dram_tensor("eoff", [128, 1], I32, kind="ExternalInput")
    encT = nc.dram_tensor("encT", [512, BT], BF16, kind="ExternalInput")
    wcT = nc.dram_tensor("wcT", [512, 256], BF16, kind="ExternalInput")
    wcb = nc.dram_tensor("wcb", [256, 1], F32, kind="ExternalInput")
    mask0 = nc.dram_tensor("mask0", [1, BT], F32, kind="ExternalInput")
    sel16 = nc.dram_tensor("sel16", [128, 16], F32, kind="ExternalInput")

    exp_out = nc.dram_tensor("exp_out", [128, VS], F32, kind="ExternalOutput")
    sum_g = nc.dram_tensor("sum_g", [128, 1], F32, kind="ExternalOutput")
    exp_c_out = nc.dram_tensor("exp_c_out", [1, BT], F32, kind="ExternalOutput")
    dh_out = nc.dram_tensor("dh_out", [128, 256], F32, kind="ExternalOutput")

    with tile.TileContext(nc) as tc:
        with (
            tc.tile_pool(name="wg", bufs=1) as wgp,
            tc.tile_pool(name="sb", bufs=2) as sb,
            tc.tile_pool(name="one", bufs=1) as one,
            tc.tile_pool(name="cst", bufs=1) as cst,
            tc.tile_pool(name="big", bufs=3, space="PSUM") as psb,
            tc.tile_pool(name="ept", bufs=3, space="PSUM") as pse,
            tc.tile_pool(name="small", bufs=2, space="PSUM") as pss,
        ):
            # ---------- GRU ----------
            w_ih_sb = cst.tile([128, 5, 768], F32R)
            nc.sync.dma_start(
                out=w_ih_sb[:],
                in_=w_ihT[:].rearrange("(a p) g -> p a g", p=128),
            )
            w_hh_sb = cst.tile([128, 2, 768], F32R)
            nc.sync.dma_start(
                out=w_hh_sb[:], in_=w_hhT[:].rearrange("(a p) g -> p a g", p=128)
            )
            bih_1 = cst.tile([1, 768], F32)
            nc.sync.dma_start(out=bih_1[:], in_=b_ih[:])
            bih_sb = cst.tile([128, 768], F32)
            nc.gpsimd.partition_broadcast(bih_sb[:], bih_1[:])
            bhh_1 = cst.tile([1, 768], F32)
            nc.sync.dma_start(out=bhh_1[:], in_=b_hh[:])
            bhh_sb = cst.tile([128, 768], F32)
            nc.gpsimd.partition_broadcast(bhh_sb[:], bhh_1[:])
            selT_sb = cst.tile([128, 4, 128], F32R)
            nc.sync.dma_start(
                out=selT_sb[:], in_=selT[:].rearrange("(a p) b -> p a b", p=128)
            )
            prevT_sb = cst.tile([128, 2, 128], F32R)
            nc.sync.dma_start(
                out=prevT_sb[:], in_=prevT[:].rearrange("(a p) b -> p a b", p=128)
            )
            prevn_sb = cst.tile([128, 256], F32)
            nc.sync.dma_start(out=prevn_sb[:], in_=prev_n[:])

            # embedded: gather emb rows -> [128, 128]; per-partition contiguous run
            eoff_sb = cst.tile([128, 1], I32)
            nc.sync.dma_start(out=eoff_sb[:], in_=eoff[:])
            embedded = sb.tile([128, 128], F32, tag="embedded")
            nc.gpsimd.indirect_dma_start(
                out=embedded[:],
                out_offset=None,
                in_=emb[:, :],
                in_offset=bass.IndirectOffsetOnAxis(ap=eoff_sb[:], axis=0),
            )
            # transpose embedded -> embT [128, 128] f32r
            embT_ps = pss.tile([128, 128], F32, space="PSUM", tag="small")
            ident = cst.tile([128, 128], F32)
            from concourse.masks import make_identity

            make_identity(nc, ident[:])
            nc.tensor.transpose(out=embT_ps[:], in_=embedded[:], identity=ident[:])
            embT = sb.tile([128, 128], F32R, tag="embT")
            nc.vector.tensor_copy(out=embT[:], in_=embT_ps[:])

            # gi = gru_inT.T @ w_ihT  (K = 640 = 4x128 sel + 1x128 emb)
            gi_ps1 = psb.tile([128, 512], F32, space="PSUM", tag="big")
            gi_ps2 = psb.tile([128, 256], F32, space="PSUM", tag="big")
            for k in range(5):
                lhs = selT_sb[:, k, :] if k < 4 else embT[:]
                nc.tensor.matmul(
                    gi_ps1[:], lhs, w_ih_sb[:, k, 0:512], start=(k == 0), stop=(k == 4)
                )
            for k in range(5):
                lhs = selT_sb[:, k, :] if k < 4 else embT[:]
                nc.tensor.matmul(
                    gi_ps2[:], lhs, w_ih_sb[:, k, 512:768], start=(k == 0), stop=(k == 4)
                )
            gi = one.tile([128, 768], F32, tag="gi")
            nc.vector.tensor_tensor(
                out=gi[:, 0:512], in0=gi_ps1[:], in1=bih_sb[:, 0:512],
                op=mybir.AluOpType.add,
            )
            nc.vector.tensor_tensor(
                out=gi[:, 512:768], in0=gi_ps2[:], in1=bih_sb[:, 512:768],
                op=mybir.AluOpType.add,
            )
            gh_ps1 = psb.tile([128, 512], F32, space="PSUM", tag="big")
            gh_ps2 = psb.tile([128, 256], F32, space="PSUM", tag="big")
            for k in range(2):
                nc.tensor.matmul(
                    gh_ps1[:], prevT_sb[:, k, :], w_hh_sb[:, k, 0:512],
                    start=(k == 0), stop=(k == 1),
                )
            for k in range(2):
                nc.tensor.matmul(
                    gh_ps2[:], prevT_sb[:, k, :], w_hh_sb[:, k, 512:768],
                    start=(k == 0), stop=(k == 1),
                )
            gh = one.tile([128, 768], F32, tag="gh")
            nc.vector.tensor_tensor(
                out=gh[:, 0:512], in0=gh_ps1[:], in1=bhh_sb[:, 0:512],
                op=mybir.AluOpType.add,
            )
            nc.vector.tensor_tensor(
                out=gh[:, 512:768], in0=gh_ps2[:], in1=bhh_sb[:, 512:768],
                op=mybir.AluOpType.add,
            )
            # gates: r, z, n  (torch order r, z, n)
            rz_in = sb.tile([128, 512], F32, tag="rzin")
            nc.vector.tensor_tensor(
                out=rz_in[:], in0=gi[:, 0:512], in1=gh[:, 0:512], op=mybir.AluOpType.add
            )
            rz = sb.tile([128, 512], F32, tag="rz")
            nc.scalar.activation(out=rz[:], in_=rz_in[:], func=mybir.ActivationFunctionType.Sigmoid)
            n_in = sb.tile([128, 256], F32, tag="nin")
            nc.vector.tensor_tensor(
                out=n_in[:], in0=rz[:, 0:256], in1=gh[:, 512:768], op=mybir.AluOpType.mult
            )
            nc.vector.tensor_tensor(
                out=n_in[:], in0=n_in[:], in1=gi[:, 512:768], op=mybir.AluOpType.add
            )
            n_t = sb.tile([128, 256], F32, tag="nt")
            nc.scalar.activation(out=n_t[:], in_=n_in[:], func=mybir.ActivationFunctionType.Tanh)
            # dh = n + z*(h - n)
            dh = sb.tile([128, 256], F32, tag="dh")
            nc.vector.tensor_tensor(
                out=dh[:], in0=prevn_sb[:], in1=n_t[:], op=mybir.AluOpType.subtract
            )
            nc.vector.tensor_tensor(
                out=dh[:], in0=dh[:], in1=rz[:, 256:512], op=mybir.AluOpType.mult
            )
            nc.vector.tensor_tensor(
                out=dh[:], in0=dh[:], in1=n_t[:], op=mybir.AluOpType.add
            )
            nc.sync.dma_start(out=dh_out[:], in_=dh[:])

            # dhT (2 k-tiles) f32r
            dhT = sb.tile([128, 2, 128], F32R, tag="dhT")
            for k in range(2):
                tp = pss.tile([128, 128], F32, space="PSUM", tag="small", name=f"dhT_ps_{k}")
                nc.tensor.transpose(out=tp[:], in_=dh[:, k * 128 : (k + 1) * 128], identity=ident[:])
                nc.vector.tensor_copy(out=dhT[:, k, :], in_=tp[:])
            # own-batch-rows dh slice via host one-hot selector, transposed, bf16
            sel_sb = cst.tile([128, 16], F32)
            nc.sync.dma_start(out=sel_sb[:], in_=sel16[:])
            dh16_ps = pss.tile([16, 256], F32, space="PSUM", tag="small")
            nc.tensor.matmul(dh16_ps[:], sel_sb[:], dh[:], start=True, stop=True)
            dh16 = sb.tile([16, 256], F32, tag="dh16")
            nc.vector.tensor_copy(out=dh16[:], in_=dh16_ps[:])
            dh16Tb = sb.tile([128, 2, 16], BF16, tag="dh16Tb")
            for k in range(2):
                tp2 = pss.tile([128, 16], F32, space="PSUM", tag="small", name=f"dh16T_ps_{k}")
                nc.tensor.transpose(out=tp2[:], in_=dh16[:, k * 128 : (k + 1) * 128], identity=ident[0:16, 0:16])
                nc.vector.tensor_copy(out=dh16Tb[:, k, :], in_=tp2[:])
            ones1f = cst.tile([1, 128], F32)
            nc.vector.memset(ones1f[:], 1.0)
            ones1 = cst.tile([1, 128], F32R)
            nc.vector.tensor_copy(out=ones1[:], in_=ones1f[:])

            # ---------- score_g: exp slab + row sums ----------
            off = 0
            sg_slices = []
            for w in SG_CHUNKS:
                sg_slices.append(slice(off, off + w))
                off += w
            sums_c = one.tile([128, len(SG_CHUNKS)], F32, tag="sums_c")
            for c, sl in enumerate(sg_slices):
                w = SG_CHUNKS[c]
                wgch = wgp.tile([128, 2, 512], F32R, tag="wgch", name=f"wgch_{c}", bufs=3)
                nc.sync.dma_start(
                    out=wgch[:, :, 0:w],
                    in_=wgt[0:256, :].rearrange("(a p) v -> p a v", p=128)[:, :, sl],
                )
                ps = psb.tile([128, SG_CHUNKS[c]], F32, space="PSUM", tag="big", name=f"sg_ps_{c}")
                nc.tensor.matmul(ps[:], dhT[:, 0, :], wgch[:, 0, 0:w], start=True, stop=False)
                nc.tensor.matmul(ps[:], dhT[:, 1, :], wgch[:, 1, 0:w], start=False, stop=False)
                wgbch = sb.tile([1, 512], F32R, tag="wgbch", name=f"wgbch_{c}", bufs=3)
                nc.sync.dma_start(out=wgbch[:, 0:w], in_=wgt[256:257, sl])
                nc.tensor.matmul(ps[:], ones1[:], wgbch[:, 0:w], start=False, stop=True)
                exps = sb.tile([128, 512], F32, tag="exps", name=f"exps_{c}", bufs=3)
                nc.scalar.activation(
                    out=exps[:, 0 : SG_CHUNKS[c]], in_=ps[:], func=mybir.ActivationFunctionType.Exp,
                    accum_out=sums_c[:, c : c + 1],
                )
                nc.sync.dma_start(out=exp_out[:, sl], in_=exps[:, 0 : SG_CHUNKS[c]])
            sumg = sb.tile([128, 1], F32, tag="sumg")
            nc.vector.reduce_sum(out=sumg[:], in_=sums_c[:], axis=mybir.AxisListType.X)
            nc.sync.dma_start(out=sum_g[:], in_=sumg[:])

            # ---------- enc side: EPT = tanh(WcT.T @ encT + b) ----------
            wc_sb = cst.tile([128, 4, 256], BF16)
            nc.sync.dma_start(
                out=wc_sb[:], in_=wcT[:].rearrange("(a p) h -> p a h", p=128)
            )
            wcb_sb = cst.tile([128, 2, 1], F32)
            nc.sync.dma_start(
                out=wcb_sb[:], in_=wcb[:].rearrange("(a p) o -> p a o", p=128)
            )
            ept = one.tile([128, 2, BT], BF16, tag="ept")
            ept_slices = [slice(sum(EPT_CHUNKS[:i]), sum(EPT_CHUNKS[: i + 1])) for i in range(len(EPT_CHUNKS))]
            for ch, esl in enumerate(ept_slices):
                w = EPT_CHUNKS[ch]
                ench = sb.tile([128, 4, 512], BF16, tag="ench", name=f"ench_{ch}", bufs=3)
                nc.sync.dma_start(
                    out=ench[:, :, 0:w],
                    in_=encT[:, esl].rearrange("(a p) t -> p a t", p=128),
                )
                for m in range(2):
                    ps = pse.tile([128, 512], F32, space="PSUM", tag="ept", name=f"ept_ps_{m}_{ch}")
                    for k in range(4):
                        nc.tensor.matmul(
                            ps[:, 0:w],
                            wc_sb[:, k, m * 128 : (m + 1) * 128],
                            ench[:, k, 0:w],
                            start=(k == 0),
                            stop=(k == 3),
                        )
                    nc.scalar.activation(
                        out=ept[:, m, esl], in_=ps[:, 0:w],
                        func=mybir.ActivationFunctionType.Tanh, bias=wcb_sb[:, m, :],
                    )
            # score_c = tanh( sum_h EPT[h, bt] * dhT[h, b(bt)] )
            for m in range(2):
                nc.vector.tensor_tensor(
                    out=ept[:, m, :].rearrange("p (b t) -> p b t", b=BS),
                    in0=ept[:, m, :].rearrange("p (b t) -> p b t", b=BS),
                    in1=dh16Tb[:, m, 0:BS][:, :, None].to_broadcast([128, BS, T]),
                    op=mybir.AluOpType.mult,
                )
            onesK = cst.tile([128, 1], BF16)
            nc.vector.memset(onesK[:], 1.0)
            scoreC = one.tile([1, BT], F32, tag="scoreC")
            for ch, sl in enumerate(ept_slices):
                ps = pss.tile([1, EPT_CHUNKS[ch]], F32, space="PSUM", tag="small")
                nc.tensor.matmul(ps[:], onesK[:], ept[:, 0, sl], start=True, stop=False)
                nc.tensor.matmul(ps[:], onesK[:], ept[:, 1, sl], start=False, stop=True)
                nc.vector.tensor_copy(out=scoreC[:, sl], in_=ps[:])
            nc.scalar.activation(out=scoreC[:], in_=scoreC[:], func=mybir.ActivationFunctionType.Tanh)
            nc.scalar.activation(out=scoreC[:], in_=scoreC[:], func=mybir.ActivationFunctionType.Exp)
            mask_sb = cst.tile([1, BT], F32)
            nc.sync.dma_start(out=mask_sb[:], in_=mask0[:])
            nc.vector.tensor_tensor(
                out=scoreC[:], in0=scoreC[:], in1=mask_sb[:], op=mybir.AluOpType.mult
            )
            nc.sync.dma_start(out=exp_c_out[:], in_=scoreC[:])
    nc.compile()
    return nc


def _build_neff2():
    nc = bacc.Bacc("TRN2", target_bir_lowering=False, debug=False, num_devices=NCORE)
    exp_in = nc.dram_tensor("exp_in", [128, VS], F32, kind="ExternalInput")
    expc_full = nc.dram_tensor("expc_full", [128, 200], F32, kind="ExternalInput")
    sumg_all = nc.dram_tensor("sumg_all", [128, 8], F32, kind="ExternalInput")
    expc_own = nc.dram_tensor("expc_own", [16, 200], F32, kind="ExternalInput")
    sumg_own = nc.dram_tensor("sumg_own", [16, 8], F32, kind="ExternalInput")
    ls_idx0 = nc.dram_tensor("ls_idx0", [4, 128, 256], I16, kind="ExternalInput")
    dup_idx = nc.dram_tensor("dup_idx", [128, 256], I16, kind="ExternalInput")
    post_oov = nc.dram_tensor("post_oov", [1, 64], F32, kind="ExternalInput")
    pam = nc.dram_tensor("pam", [16, 200], F32, kind="ExternalInput")
    rsel = nc.dram_tensor("rsel", [16, MPAD], F32, kind="ExternalInput")
    t_oh = nc.dram_tensor("t_oh", [MPAD, 200], F32, kind="ExternalInput")
    b_oh = nc.dram_tensor("b_oh", [MPAD, 16], F32, kind="ExternalInput")
    gthr_idx = nc.dram_tensor("gthr_idx", [128, 4 * MPAD], I32, kind="ExternalInput")
    encT = nc.dram_tensor("encT", [512, BT], BF16, kind="ExternalInput")

    prob_slab = nc.dram_tensor("prob_slab", [128, VS], F32, kind="ExternalOutput")
    srn_out = nc.dram_tensor("srn_out", [16, 512], F32, kind="ExternalOutput")

    with tile.TileContext(nc) as tc:
        with (
            tc.tile_pool(name="sb", bufs=2) as sb,
            tc.tile_pool(name="cst", bufs=1) as cst,
            tc.tile_pool(name="ps", bufs=2, space="PSUM") as psp,
        ):
            expc = cst.tile([128, 200], F32)
            nc.sync.dma_start(out=expc[:], in_=expc_full[:])
            sga = cst.tile([128, 8], F32)
            nc.sync.dma_start(out=sga[:], in_=sumg_all[:])
            denom = sb.tile([128, 1], F32, tag="denom")
            dtmp = sb.tile([128, 1], F32, tag="dtmp")
            nc.vector.reduce_sum(out=denom[:], in_=sga[:], axis=mybir.AxisListType.X)
            nc.vector.reduce_sum(out=dtmp[:], in_=expc[:], axis=mybir.AxisListType.X)
            nc.vector.tensor_tensor(out=denom[:], in0=denom[:], in1=dtmp[:], op=mybir.AluOpType.add)
            recip = sb.tile([128, 1], F32, tag="recip")
            nc.vector.reciprocal(out=recip[:], in_=denom[:])
            # prob_c (normalized) -> bf16 padded [128, 256]
            probc = sb.tile([128, 200], F32, tag="probc")
            nc.vector.tensor_scalar_mul(out=probc[:], in0=expc[:], scalar1=recip[:, 0:1])
            ls_data = sb.tile([128, 256], BF16, tag="ls_data")
            nc.vector.memset(ls_data[:], 0)
            nc.vector.tensor_copy(out=ls_data[:, 0:200], in_=probc[:])
            # fold duplicate (rank-1) values onto their leader slots
            didx = cst.tile([128, 256], I16)
            nc.sync.dma_start(out=didx[:], in_=dup_idx[:])
            dup_data = sb.tile([128, 256], BF16, tag="dup_data")
            nc.gpsimd.local_scatter(
                out_ap=dup_data[:], data_ap=ls_data[:], idxs_ap=didx[:],
                channels=128, num_elems=256, num_idxs=256,
            )
            nc.vector.tensor_tensor(
                out=ls_data[:], in0=ls_data[:], in1=dup_data[:], op=mybir.AluOpType.add
            )
            # scatter + normalize + emit, chunk by chunk
            lsidx_sb = cst.tile([128, 4, 256], I16)
            nc.sync.dma_start(
                out=lsidx_sb[:], in_=ls_idx0[:].rearrange("c p s -> p c s")
            )
            oov_1 = cst.tile([1, 64], F32)
            nc.sync.dma_start(out=oov_1[:], in_=post_oov[:])
            oov_sb = cst.tile([128, 64], F32)
            nc.gpsimd.partition_broadcast(oov_sb[:], oov_1[:])
            off = 0
            for c, w in enumerate(LS_CHUNKS):
                sl = slice(off, off + w)
                off += w
                expg = sb.tile([128, 1574], F32, tag="expg")
                nc.sync.dma_start(out=expg[:, 0:w], in_=exp_in[:, sl])
                scratch = sb.tile([128, 1574], BF16, tag="scratch")
                nc.gpsimd.local_scatter(
                    out_ap=scratch[:, 0:w], data_ap=ls_data[:], idxs_ap=lsidx_sb[:, c, :],
                    channels=128, num_elems=w, num_idxs=256,
                )
                outt = sb.tile([128, 1574], F32, tag="outt")
                nc.vector.scalar_tensor_tensor(
                    out=outt[:, 0:w], in0=expg[:, 0:w], scalar=recip[:, 0:1],
                    in1=scratch[:, 0:w], op0=mybir.AluOpType.mult, op1=mybir.AluOpType.add,
                )
                if sl.start <= OOV_SL[0] < sl.stop:
                    lo = OOV_SL[0] - sl.start
                    nc.vector.tensor_tensor(
                        out=outt[:, lo : lo + 64],
                        in0=outt[:, lo : lo + 64],
                        in1=oov_sb[:],
                        op=mybir.AluOpType.add,
                    )
                nc.sync.dma_start(out=prob_slab[:, sl], in_=outt[:, 0:w])

            # ---------- selective read ----------
            expco = cst.tile([16, 200], F32)
            nc.sync.dma_start(out=expco[:], in_=expc_own[:])
            sgo = cst.tile([16, 8], F32)
            nc.sync.dma_start(out=sgo[:], in_=sumg_own[:])
            dno = sb.tile([16, 1], F32, tag="dno")
            dno2 = sb.tile([16, 1], F32, tag="dno2")
            nc.vector.reduce_sum(out=dno[:], in_=sgo[:], axis=mybir.AxisListType.X)
            nc.vector.reduce_sum(out=dno2[:], in_=expco[:], axis=mybir.AxisListType.X)
            nc.vector.tensor_tensor(out=dno[:], in0=dno[:], in1=dno2[:], op=mybir.AluOpType.add)
            rcpo = sb.tile([16, 1], F32, tag="rcpo")
            nc.vector.reciprocal(out=rcpo[:], in_=dno[:])
            pam_sb = cst.tile([16, 200], F32)
            nc.sync.dma_start(out=pam_sb[:], in_=pam[:])
            pa = sb.tile([16, 200], F32, tag="pa")
            nc.vector.tensor_tensor(out=pa[:], in0=expco[:], in1=pam_sb[:], op=mybir.AluOpType.mult)
            nc.vector.tensor_scalar_mul(out=pa[:], in0=pa[:], scalar1=rcpo[:, 0:1])
            rsel_sb = cst.tile([16, MPAD], F32)
            nc.sync.dma_start(out=rsel_sb[:], in_=rsel[:])
            paM_ps = psp.tile([MPAD, 200], F32, space="PSUM", tag="ps")
            nc.tensor.matmul(paM_ps[:], rsel_sb[:], pa[:], start=True, stop=True)
            toh_sb = cst.tile([MPAD, 200], F32)
            nc.sync.dma_start(out=toh_sb[:], in_=t_oh[:])
            pav_in = sb.tile([MPAD, 200], F32, tag="pav_in")
            nc.vector.tensor_tensor(
                out=pav_in[:], in0=paM_ps[:], in1=toh_sb[:], op=mybir.AluOpType.mult
            )
            pav = sb.tile([MPAD, 1], F32, tag="pav")
            nc.vector.reduce_sum(out=pav[:], in_=pav_in[:], axis=mybir.AxisListType.X)
            boh_sb = cst.tile([MPAD, 16], F32)
            nc.sync.dma_start(out=boh_sb[:], in_=b_oh[:])
            PB = sb.tile([MPAD, 16], F32, tag="PB")
            nc.vector.tensor_scalar_mul(out=PB[:], in0=boh_sb[:], scalar1=pav[:, 0:1])
            # gather matched enc columns: encM[p, j*MPAD+m] = encT.flat[(j*128+p)*BT + bt_m]
            gi_sb = cst.tile([128, 4 * MPAD], I32)
            nc.sync.dma_start(out=gi_sb[:], in_=gthr_idx[:])
            encM = sb.tile([128, 4 * MPAD], BF16, tag="encM")
            nc.gpsimd.memset(encM[:], 0)
            nc.gpsimd.indirect_dma_start(
                out=encM[:].rearrange("p (a b) -> p a b", b=1),
                out_offset=None,
                in_=encT[:, :],
                in_offset=bass.IndirectOffsetOnAxis(ap=gi_sb[:], axis=1),
                bounds_check=512 * BT - 1,
                oob_is_err=False,
            )
            ident = cst.tile([128, 128], F32)
            from concourse.masks import make_identity

            make_identity(nc, ident[:])
            encMf = sb.tile([128, 4 * MPAD], F32, tag="encMf")
            nc.vector.tensor_copy(out=encMf[:], in_=encM[:])
            srn_sb = sb.tile([16, 512], F32, tag="srn_sb")
            for j in range(4):
                tps = psp.tile([MPAD, 128], F32, space="PSUM", tag="ps", name=f"tps_{j}")
                nc.tensor.transpose(
                    out=tps[:], in_=encMf[:, j * MPAD : (j + 1) * MPAD], identity=ident[:]
                )
                encMT = sb.tile([MPAD, 128], F32, tag="encMT")
                nc.vector.tensor_copy(out=encMT[:], in_=tps[:])
                sps = psp.tile([128, 16], F32, space="PSUM", tag="ps", name=f"sps_{j}")
                nc.tensor.matmul(sps[:], encMT[:], PB[:], start=True, stop=True)
                ssb = sb.tile([128, 16], F32, tag="ssb", name=f"ssb_{j}")
                nc.vector.tensor_copy(out=ssb[:], in_=sps[:])
                tps2 = psp.tile([16, 128], F32, space="PSUM", tag="ps", name=f"tps2_{j}")
                nc.tensor.transpose(out=tps2[:], in_=ssb[:], identity=ident[:])
                nc.vector.tensor_copy(out=srn_sb[:, j * 128 : (j + 1) * 128], in_=tps2[:])
            nc.sync.dma_start(out=srn_out[:], in_=srn_sb[:])
    nc.compile()
    return nc


def _get_neffs():
    if "n1" not in _CACHE:
        _CACHE["n1"] = _build_neff1()
        _CACHE["n2"] = _build_neff2()
    return _CACHE["n1"], _CACHE["n2"]


def profile_hw(inputs):
    """Run both NEFFs with NTFF tracing; return summed exec_time_ns."""
    _, t1, t2 = _run(trace=True, **inputs)
    return (t1 or 0) + (t2 or 0)


def kernel(**inputs):
    out, _, _ = _run(trace=False, **inputs)
    return out


def _run(
    decoder_input, encoder_outputs, encoder_idxs, prev_state, selective_read, step,
    emb, W_ih, W_hh, b_ih, b_hh, Wi_w, Wi_b, Wg_w, Wg_b, Wc_w, Wc_b, trace=False,
):
    assert int(step) == 1, "kernel specialized for step != 0 path"
    di = np.asarray(decoder_input).astype(np.int64)
    idx = np.asarray(encoder_idxs).astype(np.int64)
    enc = np.asarray(encoder_outputs, dtype=np.float32)
    n1, n2 = _get_neffs()

    # ---- host staging (layout transforms + integer index metadata only) ----
    WgT_ext = np.full((257, VPAD), 0.0, np.float32)
    WgT_ext[0:256, 0:V] = np.asarray(Wg_w, np.float32).T
    WgT_ext[256, 0:V] = np.asarray(Wg_b, np.float32)
    WgT_ext[256, V:] = -10000.0
    w_ihT = np.ascontiguousarray(np.asarray(W_ih, np.float32).T)
    w_hhT = np.ascontiguousarray(np.asarray(W_hh, np.float32).T)
    selT = np.ascontiguousarray(np.asarray(selective_read, np.float32)[:, 0, :].T)
    prevT = np.ascontiguousarray(np.asarray(prev_state, np.float32)[0].T)
    prev_n = np.ascontiguousarray(np.asarray(prev_state, np.float32)[0])
    wcT = np.ascontiguousarray(np.asarray(Wc_w, np.float32).T).astype(ml_dtypes.bfloat16)
    wcb = np.asarray(Wc_b, np.float32).reshape(256, 1)
    eoff = di[:, 0].astype(np.int32).reshape(128, 1)
    emb2d = np.ascontiguousarray(np.asarray(emb, np.float32))

    in1_maps = []
    encTs = []
    for k in range(NCORE):
        bset = slice(k * BS, (k + 1) * BS)
        encT_k = np.ascontiguousarray(
            enc[bset].transpose(2, 0, 1).reshape(512, BT)
        ).astype(ml_dtypes.bfloat16)
        encTs.append(encT_k)
        mask0 = (idx[bset] != 0).astype(np.float32).reshape(1, BT)
        sel16_a = np.zeros((128, 16), np.float32)
        for j in range(BS):
            sel16_a[k * BS + j, j] = 1.0
        in1_maps.append({
            "wgt": np.ascontiguousarray(WgT_ext[:, k * VS : (k + 1) * VS]),
            "w_ihT": w_ihT, "w_hhT": w_hhT,
            "b_ih": np.asarray(b_ih, np.float32).reshape(1, 768),
            "b_hh": np.asarray(b_hh, np.float32).reshape(1, 768),
            "selT": selT, "prevT": prevT, "prev_n": prev_n,
            "emb": emb2d, "eoff": eoff,
            "encT": encT_k, "wcT": wcT, "wcb": wcb, "mask0": mask0, "sel16": sel16_a,
        })

    res1 = run_bass_kernel_spmd(n1, in1_maps, core_ids=list(range(NCORE)), trace=trace)
    r1 = res1.results

    # ---- between-kernel glue: pure concatenation / slicing ----
    expc_full = np.concatenate(
        [r1[k]["exp_c_out"].reshape(BS, T) for k in range(NCORE)], axis=0
    )  # [128, 200]
    sumg_all = np.stack([r1[k]["sum_g"][:, 0] for k in range(NCORE)], axis=1)  # [128, 8]
    dh = r1[0]["dh_out"]

    # ---- scatter metadata (integer bookkeeping) ----
    owner = idx // VS          # [128, 200]
    local = (idx - owner * VS).astype(np.int64)
    # occurrence rank of (b, idx-value)
    rank = np.zeros((B, T), np.int64)
    for b in range(B):
        seen = {}
        for t in range(T):
            v = idx[b, t]
            r = seen.get(v, 0)
            rank[b, t] = r
            seen[v] = r + 1
    assert rank.max() <= 1, f"max duplicate multiplicity {rank.max()+1} > 2 unsupported"
    # matches for selective read
    match = idx == di  # [128, 200]
    tot = match.sum(axis=1)

    ls_bounds = np.cumsum([0] + LS_CHUNKS)
    in2_maps = []
    for k in range(NCORE):
        bset = slice(k * BS, (k + 1) * BS)
        own = owner == k
        lsi = np.full((4, 128, 256), -1, np.int16)
        for c in range(4):
            m = own & (rank == 0) & (local >= ls_bounds[c]) & (local < ls_bounds[c + 1])
            bb, tt = np.nonzero(m)
            lsi[c, bb, tt] = (local[bb, tt] - ls_bounds[c]).astype(np.int16)
        dupi = np.full((128, 256), -1, np.int16)
        m = own & (rank == 1)
        bb, tt = np.nonzero(m)
        for b_, t_ in zip(bb, tt):
            tlead = np.nonzero((idx[b_] == idx[b_, t_]) & (rank[b_] == 0))[0][0]
            dupi[b_, t_] = tlead
        post_oov = np.zeros((1, 64), np.float32)
        if k == 7:
            lo = OOV_SL[0]
            for j in range(64):
                col = 7 * VS + lo + j
                if V <= col < V + OOV:
                    post_oov[0, j] = 1e-4
        # selective-read metadata for this core's rows
        mrows = [(b_ - k * BS, t_) for b_, t_ in zip(*np.nonzero(match[bset]))]
        assert len(mrows) <= MPAD, f"too many matches {len(mrows)}"
        pam = np.zeros((16, 200), np.float32)
        for bl, t_ in mrows:
            pam[bl, t_] = 1.0 / max(tot[k * BS + bl], 1)
        rsel_a = np.zeros((16, MPAD), np.float32)
        toh = np.zeros((MPAD, 200), np.float32)
        boh = np.zeros((MPAD, 16), np.float32)
        gi_a = np.full((128, 4 * MPAD), 2**28, np.int32)
        for m_i, (bl, t_) in enumerate(mrows):
            rsel_a[bl, m_i] = 1.0
            toh[m_i, t_] = 1.0
            boh[m_i, bl] = 1.0
        # gather stream mapping: out[p, f] <- idx_hw[s % 128, s // 128], s = p*F + f
        Fw = 4 * MPAD
        for m_i, (bl, t_) in enumerate(mrows):
            bt = bl * T + t_
            for j in range(4):
                for p in range(128):
                    s = p * Fw + j * MPAD + m_i
                    gi_a[s % 128, s // 128] = (j * 128 + p) * BT + bt
        in2_maps.append({
            "exp_in": r1[k]["exp_out"],
            "expc_full": expc_full.astype(np.float32),
            "sumg_all": sumg_all.astype(np.float32),
            "expc_own": expc_full[bset].astype(np.float32),
            "sumg_own": sumg_all[bset].astype(np.float32),
            "ls_idx0": lsi, "dup_idx": dupi, "post_oov": post_oov,
            "pam": pam, "rsel": rsel_a, "t_oh": toh, "b_oh": boh,
            "gthr_idx": gi_a,
            "encT": encTs[k],
        })

    res2 = run_bass_kernel_spmd(n2, in2_maps, core_ids=list(range(NCORE)), trace=trace)
    r2 = res2.results

    slab = np.concatenate([r2[k]["prob_slab"] for k in range(NCORE)], axis=1)  # [128, VPAD]
    prob_out = slab[:, : V + OOV].reshape(B, 1, V + OOV).astype(np.float32)
    srn = np.concatenate([r2[k]["srn_out"] for k in range(NCORE)], axis=0).reshape(B, 1, 512)
    out = (prob_out, dh.astype(np.float32), srn.astype(np.float32))
    return out, res1.exec_time_ns, res2.exec_time_ns
